# revision 24
# baseline (speedup 1.0000x reference)
"""EdgePredictionHead on 8 TRN2 NeuronCores.

Key structure exploited:

1. The reference output is exactly symmetric in (i, j): e_sym, the squared
   distance d, and s[i]+s[j] are all symmetric, so bonds_pred for edge (j,i)
   equals bonds_pred for (i,j). We compute only the 496 half-edges (j<i)
   per molecule and mirror on the host -> halves all device work.

2. The edge MLP's first layer is affine, so the host folds W_b0 into the
   other weights:  pre = e_sym @ (W_bond@W0) + G + b_eff  with
   G = a[i]+a[j]+d*w_d,  a = silu(s@W_shared+b_shared) @ W0.  G is cheap to
   build on the host and ships as fp16; b_eff rides as the scalar-engine
   activation bias.

Device pipeline per 496-edge chunk (one molecule), per 128-row half:

    DVE   copies G.T half into the PSUM bank (fp16 -> fp32)
    PE    matmul accumulates W_bond0_half.T @ esymT on top (start=False)
    ACT   h = silu(psum + b_eff_half)            (PSUM -> SBUF fp16)
    PE    outT[5,496] = wb1a.T @ h0 + wb1b.T @ h1
    DVE/ACT copy outT into an SBUF staging tile; one DMA at the end.

Input DMAs are spread across the SP and GpSimd sequencers (separate DGE
queues) so descriptor issue isn't serialized on one engine.  Everything on
device is fp16 (PSUM accumulates fp32); rel err vs the fp32 reference is
~6e-4, far under the 2e-2 gate.
"""

import os
import sys
import numpy as np

sys.path.insert(0, "/opt/trn_rl_repo")

import concourse.bass as bass
import concourse.mybir as mybir
from concourse import bacc
from concourse.tile import TileContext
from concourse.bass_utils import run_bass_kernel_spmd

N_CORES = 8
SDIM = 256
EDIM = 128
NB = 5
ATOMS = 32
NUM_MOL = 32
MOL_PER_CORE = NUM_MOL // N_CORES          # 4
HP = ATOMS * (ATOMS - 1) // 2              # 496 half-edges per molecule
E_LOC = MOL_PER_CORE * HP                  # 1984 per core
CW = 3 * HP                                # 1488 cols per chunk in `eg`

F32 = mybir.dt.float32
F16 = mybir.dt.float16

_nc_cache = {}


def _build_nc():
    if "nc" in _nc_cache:
        return _nc_cache["nc"]
    nc = bacc.Bacc()
    wt = nc.dram_tensor("wt", [128, 266], F16, kind="ExternalInput")
    bias = nc.dram_tensor("bias", [128, 2], F32, kind="ExternalInput")
    # per chunk [128, 1488]: [esymT 496 | G.T rows 0:128 | G.T rows 128:256]
    eg = nc.dram_tensor("eg", [128, MOL_PER_CORE * CW], F16,
                        kind="ExternalInput")
    outT = nc.dram_tensor("outT", [NB, E_LOC], F32, kind="ExternalOutput")

    Silu = mybir.ActivationFunctionType.Silu
    with TileContext(nc) as tc:
        with tc.tile_pool(name="const", bufs=1) as cpool, \
             tc.tile_pool(name="io", bufs=4) as bpool, \
             tc.tile_pool(name="work", bufs=4) as wpool, \
             tc.tile_pool(name="pp", bufs=2, space="PSUM") as ppool, \
             tc.tile_pool(name="po", bufs=2, space="PSUM") as opool:
            wt_t = cpool.tile([128, 266], F16)
            bias_t = cpool.tile([128, 2], F32)
            out_sb = cpool.tile([NB, E_LOC], F32)

            # Input DMAs spread over three DGE queues (SP / ACT / GpSimd-SWDGE)
            # so descriptor streams run in parallel; chunk 0 is split across
            # SP and ACT so compute can start as early as possible.
            eg_ts = []
            for _m in range(MOL_PER_CORE):
                eg_t = bpool.tile([128, CW], F16, tag="eg")
                eg_ts.append(eg_t)
            nc.sync.dma_start(out=eg_ts[0][:, 0:2 * HP],
                              in_=eg[:, 0:2 * HP])          # esym0 | G0.h0
            nc.scalar.dma_start(out=wt_t[:], in_=wt[:])
            nc.scalar.dma_start(out=bias_t[:], in_=bias[:])
            nc.scalar.dma_start(out=eg_ts[0][:, 2 * HP:CW],
                                in_=eg[:, 2 * HP:CW])       # G0.h1
            nc.gpsimd.dma_start(out=eg_ts[3][:], in_=eg[:, 3 * CW:4 * CW])
            nc.sync.dma_start(out=eg_ts[1][:], in_=eg[:, CW:2 * CW])
            nc.scalar.dma_start(out=eg_ts[2][:], in_=eg[:, 2 * CW:3 * CW])

            # one-time warmups: sync ACT/DVE with const DMAs + load Silu table
            wu_a = cpool.tile([128, 1], F32)
            nc.scalar.activation(wu_a[:], bias_t[:, 0:1], Silu,
                                 bias=bias_t[:, 1:2])
            wu_d = cpool.tile([128, 1], F16)
            nc.vector.tensor_copy(wu_d[:], wt_t[:, 0:1])

            hs = {}

            def in_stage(m):
                eg_t = eg_ts[m]
                for hf in range(2):
                    ps = ppool.tile([128, HP], F32, tag=f"ps{hf}")
                    nc.vector.tensor_copy(
                        ps[:], eg_t[:, HP + hf * HP:HP + (hf + 1) * HP])
                    nc.tensor.matmul(
                        ps[:], wt_t[:, hf * 128:(hf + 1) * 128],
                        eg_t[:, 0:HP], start=False, stop=True,
                        skip_group_check=True)
                    h_t = wpool.tile([128, HP], F16, tag=f"h{hf}")
                    nc.scalar.activation(h_t[:], ps[:], Silu,
                                         bias=bias_t[:, hf:hf + 1])
                    hs[(m, hf)] = h_t

            def out_stage(m):
                psO = opool.tile([128, HP], F32, tag="o")
                nc.tensor.matmul(psO[0:NB, :], wt_t[:, 256:261],
                                 hs[(m, 0)][:], start=True, stop=False)
                nc.tensor.matmul(psO[0:NB, :], wt_t[:, 261:266],
                                 hs[(m, 1)][:], start=False, stop=True)
                sl = slice(m * HP, (m + 1) * HP)
                nc.vector.tensor_copy(out_sb[:, sl], psO[0:NB, :])

            # software-pipelined emission: chunk m's input stage is emitted
            # before chunk m-1's output stage, so no engine's in-order
            # stream stalls on a cross-stage dependency. Output DMAs go out
            # in two halves so the first fires mid-kernel.
            for m in range(MOL_PER_CORE):
                in_stage(m)
                if m >= 1:
                    out_stage(m - 1)
            out_stage(MOL_PER_CORE - 1)
            nc.sync.dma_start(out=outT[:, 0:2 * HP], in_=out_sb[:, 0:2 * HP])
            nc.sync.dma_start(out=outT[:, 2 * HP:E_LOC],
                              in_=out_sb[:, 2 * HP:E_LOC])

    nc.finalize()   # Bacc compile pipeline: event-sem split, act tables, ...

    # Drop the vestigial set-0 act-table load the table pass puts at block
    # entry: every activation here is Silu, whose own load follows before
    # the first InstActivation. Saves a 1.28us ACT_TABLE_LOAD.
    for b in nc.m.functions[0].blocks:
        insts = list(b.instructions)
        loads = [i for i in insts
                 if type(i).__name__ == 'InstLoadActFuncSet']
        if len(loads) >= 2 and not loads[0].sync_info:
            k1 = insts.index(loads[1])
            first_act = next((k for k, i in enumerate(insts)
                              if type(i).__name__ == 'InstActivation'),
                             None)
            if first_act is not None and k1 < first_act:
                b.instructions = [i for i in insts if i is not loads[0]]

    _nc_cache["nc"] = nc
    return nc


def _silu(x):
    return x / (1.0 + np.exp(-x))


def _structured(batch, edge_index, n, E):
    """True iff inputs follow the fully-connected per-molecule pattern."""
    if n != NUM_MOL * ATOMS or E != NUM_MOL * ATOMS * (ATOMS - 1):
        return False
    if not np.array_equal(batch,
                          np.repeat(np.arange(NUM_MOL, dtype=batch.dtype),
                                    ATOMS)):
        return False
    jj, ii = np.meshgrid(np.arange(ATOMS), np.arange(ATOMS), indexing="ij")
    msk = jj != ii
    offs = (np.arange(NUM_MOL) * ATOMS)[:, None]
    exp_j = (jj[msk][None, :] + offs).ravel()
    exp_i = (ii[msk][None, :] + offs).ravel()
    return (np.array_equal(edge_index[0], exp_j)
            and np.array_equal(edge_index[1], exp_i))


def _host_general(s, v, p, e, batch, edge_index,
                  W_shared, b_shared, W_coords, W_bond, b_bond,
                  W_b0, b_b0, W_b1, b_b1):
    """Generic fp32 host path for inputs that don't match the structure."""
    n = s.shape[0]
    E = edge_index.shape[1]
    j, i = edge_index[0].astype(np.int64), edge_index[1].astype(np.int64)
    s1 = _silu(s @ W_shared + b_shared)
    coords = p + (v @ W_coords).reshape(n, 3)
    nmol = int(batch.max()) + 1
    sums = np.zeros((nmol, 3), np.float32)
    np.add.at(sums, batch, coords)
    counts = np.maximum(np.bincount(batch, minlength=nmol), 1).astype(np.float32)
    coords = coords - (sums / counts[:, None])[batch]
    d = ((coords[i] - coords[j]) ** 2).sum(-1).astype(np.float32)
    key = j * n + i
    order = np.argsort(key)
    skey = key[order]
    pos = np.clip(np.searchsorted(skey, i * n + j), 0, E - 1)
    rev = order[pos]
    has_rev = skey[pos] == i * n + j
    e_rev = np.where(has_rev[:, None], e[rev], 0.0).astype(np.float32)
    e_sym = 0.5 * (e + e_rev)
    f = s1[i] + s1[j] + (e_sym @ W_bond + b_bond)
    h = _silu(np.concatenate([f, d[:, None]], -1) @ W_b0 + b_b0)
    return (h @ W_b1 + b_b1).astype(np.float32)


def kernel(s, v, p, e, batch, edge_index,
           W_shared, b_shared, W_coords, W_bond, b_bond,
           W_b0, b_b0, W_b1, b_b1):
    s = np.asarray(s, np.float32)
    v = np.asarray(v, np.float32)
    p = np.asarray(p, np.float32)
    e = np.asarray(e, np.float32)
    batch = np.asarray(batch, np.int32)
    edge_index = np.asarray(edge_index, np.int32)
    W_shared = np.asarray(W_shared, np.float32)
    b_shared = np.asarray(b_shared, np.float32)
    W_coords = np.asarray(W_coords, np.float32)
    W_bond = np.asarray(W_bond, np.float32)
    b_bond = np.asarray(b_bond, np.float32)
    W_b0 = np.asarray(W_b0, np.float32)
    b_b0 = np.asarray(b_b0, np.float32)
    W_b1 = np.asarray(W_b1, np.float32)
    b_b1 = np.asarray(b_b1, np.float32)
    n = s.shape[0]
    E = edge_index.shape[1]

    if not _structured(batch, edge_index, n, E):
        return _host_general(s, v, p, e, batch, edge_index,
                             W_shared, b_shared, W_coords, W_bond, b_bond,
                             W_b0, b_b0, W_b1, b_b1)

    # ---- host: node-level prep (cheap) + weight folding ----
    s1 = _silu(s @ W_shared + b_shared)                       # [n, SDIM]
    W0 = W_b0[:SDIM]                                          # [SDIM, SDIM]
    w_d = W_b0[SDIM]                                          # [SDIM]
    a = (s1 @ W0).astype(np.float32)                          # [n, SDIM]
    W_bond0 = (W_bond @ W0).astype(np.float32)                # [EDIM, SDIM]
    b_eff = (b_bond @ W0 + b_b0).astype(np.float32)           # [SDIM]

    coords = p + (v @ W_coords).reshape(n, 3)
    coords = coords - coords.reshape(NUM_MOL, ATOMS, 3).mean(1)[batch]

    # half-edge enumeration (j < i), j-major — 496 per molecule
    jl, il = np.triu_indices(ATOMS, k=1)
    fwd_local = jl * 31 + il - 1            # id of (j,i) in the full list
    mir_local = il * 31 + jl                # id of (i,j)

    mols = np.arange(NUM_MOL)
    gj = (mols[:, None] * ATOMS + jl).ravel()         # [NUM_MOL*HP]
    gi = (mols[:, None] * ATOMS + il).ravel()
    d_half = ((coords[gi] - coords[gj]) ** 2).sum(-1).astype(np.float32)
    fwd_ids = (mols[:, None] * (ATOMS * (ATOMS - 1)) + fwd_local).ravel()
    mir_ids = (mols[:, None] * (ATOMS * (ATOMS - 1)) + mir_local).ravel()
    e_sym_half = (0.5 * (e[fwd_ids] + e[mir_ids])).astype(np.float16)
    G_half = (a[gi] + a[gj]
              + d_half[:, None] * w_d[None, :]).astype(np.float16)

    wt = np.zeros((128, 266), np.float16)
    wt[:, 0:256] = W_bond0
    wt[:, 256:261] = W_b1[0:128]
    wt[:, 261:266] = W_b1[128:256]
    bias = np.stack([b_eff[0:128], b_eff[128:256]], axis=1).astype(np.float32)

    in_maps = []
    for c in range(N_CORES):
        eg = np.zeros((128, MOL_PER_CORE * CW), np.float16)
        for mloc in range(MOL_PER_CORE):
            m = c * MOL_PER_CORE + mloc
            b0 = mloc * CW
            esl = slice(m * HP, (m + 1) * HP)
            eg[:, b0:b0 + HP] = e_sym_half[esl].T
            GT = G_half[esl].T                          # [256, HP]
            eg[:, b0 + HP:b0 + 2 * HP] = GT[0:128]
            eg[:, b0 + 2 * HP:b0 + 3 * HP] = GT[128:256]
        in_maps.append({"wt": wt, "bias": bias, "eg": eg})

    try:
        nc = _build_nc()
        # The axon loopback relay can start the NEFF while the host->device
        # input copy is still streaming (first execution reads partially
        # written DRAM). Run until two consecutive executions agree — by the
        # second run the buffers are stable, so this normally costs exactly
        # two executions.
        prev = None
        for _attempt in range(6):
            res = run_bass_kernel_spmd(nc, in_maps,
                                       core_ids=list(range(N_CORES)))
            cur = [res.results[c]["outT"] for c in range(N_CORES)]
            if prev is not None and all(
                    np.array_equal(a, b) for a, b in zip(prev, cur)):
                break
            prev = cur
        globals()["LAST_RES"] = res
        results = res.results if hasattr(res, "results") else res
        out = np.zeros((E, NB), np.float32)
        for c in range(N_CORES):
            oh = results[c]["outT"].T.astype(np.float32) + b_b1  # [E_LOC, NB]
            lo, hi = c * E_LOC, (c + 1) * E_LOC
            out[fwd_ids[lo:hi]] = oh
            out[mir_ids[lo:hi]] = oh
        return out
    except Exception:
        import traceback
        traceback.print_exc()
        return _host_general(s, v, p, e, batch, edge_index,
                             W_shared, b_shared, W_coords, W_bond, b_bond,
                             W_b0, b_b0, W_b1, b_b1)


# revision 26
# speedup vs baseline: 1.0292x; 1.0292x over previous
"""EdgePredictionHead on 8 TRN2 NeuronCores.

Key structure exploited:

1. The reference output is exactly symmetric in (i, j): e_sym, the squared
   distance d, and s[i]+s[j] are all symmetric, so bonds_pred for edge (j,i)
   equals bonds_pred for (i,j). We compute only the 496 half-edges (j<i)
   per molecule and mirror on the host -> halves all device work.

2. The edge MLP's first layer is affine, so the host folds W_b0 into the
   other weights:  pre = e_sym @ (W_bond@W0) + G + b_eff  with
   G = a[i]+a[j]+d*w_d,  a = silu(s@W_shared+b_shared) @ W0.  G is cheap to
   build on the host and ships as fp16; b_eff rides as the scalar-engine
   activation bias.

Device pipeline per 496-edge chunk (one molecule), per 128-row half:

    DVE   copies G.T half into the PSUM bank (fp16 -> fp32)
    PE    matmul accumulates W_bond0_half.T @ esymT on top (start=False)
    ACT   h = silu(psum + b_eff_half)            (PSUM -> SBUF fp16)
    PE    outT[5,496] = wb1a.T @ h0 + wb1b.T @ h1
    DVE/ACT copy outT into an SBUF staging tile; one DMA at the end.

Input DMAs are spread across the SP and GpSimd sequencers (separate DGE
queues) so descriptor issue isn't serialized on one engine.  Everything on
device is fp16 (PSUM accumulates fp32); rel err vs the fp32 reference is
~6e-4, far under the 2e-2 gate.
"""

import os
import sys
import numpy as np

sys.path.insert(0, "/opt/trn_rl_repo")

import concourse.bass as bass
import concourse.mybir as mybir
from concourse import bacc
from concourse.tile import TileContext
from concourse.bass_utils import run_bass_kernel_spmd

N_CORES = 8
SDIM = 256
EDIM = 128
NB = 5
ATOMS = 32
NUM_MOL = 32
MOL_PER_CORE = NUM_MOL // N_CORES          # 4
HP = ATOMS * (ATOMS - 1) // 2              # 496 half-edges per molecule
E_LOC = MOL_PER_CORE * HP                  # 1984 per core
CW = 3 * HP                                # 1488 cols per chunk in `eg`

F32 = mybir.dt.float32
F16 = mybir.dt.float16

_nc_cache = {}


def _build_nc():
    if "nc" in _nc_cache:
        return _nc_cache["nc"]
    nc = bacc.Bacc()
    wt = nc.dram_tensor("wt", [128, 266], F16, kind="ExternalInput")
    bias = nc.dram_tensor("bias", [128, 2], F32, kind="ExternalInput")
    # per chunk [128, 1488]: [esymT 496 | G.T rows 0:128 | G.T rows 128:256]
    eg = nc.dram_tensor("eg", [128, MOL_PER_CORE * CW], F16,
                        kind="ExternalInput")
    outT = nc.dram_tensor("outT", [NB, E_LOC], F32, kind="ExternalOutput")

    Silu = mybir.ActivationFunctionType.Silu
    with TileContext(nc) as tc:
        with tc.tile_pool(name="const", bufs=1) as cpool, \
             tc.tile_pool(name="io", bufs=4) as bpool, \
             tc.tile_pool(name="work", bufs=4) as wpool, \
             tc.tile_pool(name="pp", bufs=2, space="PSUM") as ppool, \
             tc.tile_pool(name="po", bufs=2, space="PSUM") as opool:
            wt_t = cpool.tile([128, 266], F16)
            bias_t = cpool.tile([128, 2], F32)
            out_sb = cpool.tile([NB, E_LOC], F32)

            # The HWDGE queue cost is ~19ns per descriptor ROW, so pair two
            # chunks per DMA (5952B rows) to double effective bandwidth:
            # SP carries chunks 0-1, ACT carries chunks 2-3, and the small
            # consts ride the otherwise-idle GpSimd SWDGE queue.
            egp0 = bpool.tile([128, 2 * CW], F16, tag="egp")
            egp1 = bpool.tile([128, 2 * CW], F16, tag="egp")
            nc.sync.dma_start(out=egp0[:], in_=eg[:, 0:2 * CW])
            nc.scalar.dma_start(out=egp1[:], in_=eg[:, 2 * CW:4 * CW])
            nc.gpsimd.dma_start(out=wt_t[:], in_=wt[:])
            nc.gpsimd.dma_start(out=bias_t[:], in_=bias[:])
            eg_ts = [egp0, egp0, egp1, egp1]

            # one-time warmups: sync ACT/DVE with const DMAs + load Silu table
            wu_a = cpool.tile([128, 1], F32)
            nc.scalar.activation(wu_a[:], bias_t[:, 0:1], Silu,
                                 bias=bias_t[:, 1:2])
            wu_d = cpool.tile([128, 1], F16)
            nc.vector.tensor_copy(wu_d[:], wt_t[:, 0:1])

            hs = {}

            def in_stage(m):
                eg_t = eg_ts[m]
                c0 = (m % 2) * CW
                for hf in range(2):
                    ps = ppool.tile([128, HP], F32, tag=f"ps{hf}")
                    nc.vector.tensor_copy(
                        ps[:],
                        eg_t[:, c0 + HP + hf * HP:c0 + HP + (hf + 1) * HP])
                    nc.tensor.matmul(
                        ps[:], wt_t[:, hf * 128:(hf + 1) * 128],
                        eg_t[:, c0:c0 + HP], start=False, stop=True,
                        skip_group_check=True)
                    h_t = wpool.tile([128, HP], F16, tag=f"h{hf}")
                    nc.scalar.activation(h_t[:], ps[:], Silu,
                                         bias=bias_t[:, hf:hf + 1])
                    hs[(m, hf)] = h_t

            def out_stage(m):
                psO = opool.tile([128, HP], F32, tag="o")
                nc.tensor.matmul(psO[0:NB, :], wt_t[:, 256:261],
                                 hs[(m, 0)][:], start=True, stop=False)
                nc.tensor.matmul(psO[0:NB, :], wt_t[:, 261:266],
                                 hs[(m, 1)][:], start=False, stop=True)
                sl = slice(m * HP, (m + 1) * HP)
                nc.vector.tensor_copy(out_sb[:, sl], psO[0:NB, :])

            # software-pipelined emission: chunk m's input stage is emitted
            # before chunk m-1's output stage, so no engine's in-order
            # stream stalls on a cross-stage dependency. Output DMAs go out
            # in two halves so the first fires mid-kernel.
            for m in range(MOL_PER_CORE):
                in_stage(m)
                if m >= 1:
                    out_stage(m - 1)
            out_stage(MOL_PER_CORE - 1)
            nc.sync.dma_start(out=outT[:, 0:2 * HP], in_=out_sb[:, 0:2 * HP])
            nc.sync.dma_start(out=outT[:, 2 * HP:E_LOC],
                              in_=out_sb[:, 2 * HP:E_LOC])

    nc.finalize()   # Bacc compile pipeline: event-sem split, act tables, ...

    # Drop the vestigial set-0 act-table load the table pass puts at block
    # entry: every activation here is Silu, whose own load follows before
    # the first InstActivation. Saves a 1.28us ACT_TABLE_LOAD.
    for b in nc.m.functions[0].blocks:
        insts = list(b.instructions)
        loads = [i for i in insts
                 if type(i).__name__ == 'InstLoadActFuncSet']
        if len(loads) >= 2 and not loads[0].sync_info:
            k1 = insts.index(loads[1])
            first_act = next((k for k, i in enumerate(insts)
                              if type(i).__name__ == 'InstActivation'),
                             None)
            if first_act is not None and k1 < first_act:
                b.instructions = [i for i in insts if i is not loads[0]]

    _nc_cache["nc"] = nc
    return nc


def _silu(x):
    return x / (1.0 + np.exp(-x))


def _structured(batch, edge_index, n, E):
    """True iff inputs follow the fully-connected per-molecule pattern."""
    if n != NUM_MOL * ATOMS or E != NUM_MOL * ATOMS * (ATOMS - 1):
        return False
    if not np.array_equal(batch,
                          np.repeat(np.arange(NUM_MOL, dtype=batch.dtype),
                                    ATOMS)):
        return False
    jj, ii = np.meshgrid(np.arange(ATOMS), np.arange(ATOMS), indexing="ij")
    msk = jj != ii
    offs = (np.arange(NUM_MOL) * ATOMS)[:, None]
    exp_j = (jj[msk][None, :] + offs).ravel()
    exp_i = (ii[msk][None, :] + offs).ravel()
    return (np.array_equal(edge_index[0], exp_j)
            and np.array_equal(edge_index[1], exp_i))


def _host_general(s, v, p, e, batch, edge_index,
                  W_shared, b_shared, W_coords, W_bond, b_bond,
                  W_b0, b_b0, W_b1, b_b1):
    """Generic fp32 host path for inputs that don't match the structure."""
    n = s.shape[0]
    E = edge_index.shape[1]
    j, i = edge_index[0].astype(np.int64), edge_index[1].astype(np.int64)
    s1 = _silu(s @ W_shared + b_shared)
    coords = p + (v @ W_coords).reshape(n, 3)
    nmol = int(batch.max()) + 1
    sums = np.zeros((nmol, 3), np.float32)
    np.add.at(sums, batch, coords)
    counts = np.maximum(np.bincount(batch, minlength=nmol), 1).astype(np.float32)
    coords = coords - (sums / counts[:, None])[batch]
    d = ((coords[i] - coords[j]) ** 2).sum(-1).astype(np.float32)
    key = j * n + i
    order = np.argsort(key)
    skey = key[order]
    pos = np.clip(np.searchsorted(skey, i * n + j), 0, E - 1)
    rev = order[pos]
    has_rev = skey[pos] == i * n + j
    e_rev = np.where(has_rev[:, None], e[rev], 0.0).astype(np.float32)
    e_sym = 0.5 * (e + e_rev)
    f = s1[i] + s1[j] + (e_sym @ W_bond + b_bond)
    h = _silu(np.concatenate([f, d[:, None]], -1) @ W_b0 + b_b0)
    return (h @ W_b1 + b_b1).astype(np.float32)


def kernel(s, v, p, e, batch, edge_index,
           W_shared, b_shared, W_coords, W_bond, b_bond,
           W_b0, b_b0, W_b1, b_b1):
    s = np.asarray(s, np.float32)
    v = np.asarray(v, np.float32)
    p = np.asarray(p, np.float32)
    e = np.asarray(e, np.float32)
    batch = np.asarray(batch, np.int32)
    edge_index = np.asarray(edge_index, np.int32)
    W_shared = np.asarray(W_shared, np.float32)
    b_shared = np.asarray(b_shared, np.float32)
    W_coords = np.asarray(W_coords, np.float32)
    W_bond = np.asarray(W_bond, np.float32)
    b_bond = np.asarray(b_bond, np.float32)
    W_b0 = np.asarray(W_b0, np.float32)
    b_b0 = np.asarray(b_b0, np.float32)
    W_b1 = np.asarray(W_b1, np.float32)
    b_b1 = np.asarray(b_b1, np.float32)
    n = s.shape[0]
    E = edge_index.shape[1]

    if not _structured(batch, edge_index, n, E):
        return _host_general(s, v, p, e, batch, edge_index,
                             W_shared, b_shared, W_coords, W_bond, b_bond,
                             W_b0, b_b0, W_b1, b_b1)

    # ---- host: node-level prep (cheap) + weight folding ----
    s1 = _silu(s @ W_shared + b_shared)                       # [n, SDIM]
    W0 = W_b0[:SDIM]                                          # [SDIM, SDIM]
    w_d = W_b0[SDIM]                                          # [SDIM]
    a = (s1 @ W0).astype(np.float32)                          # [n, SDIM]
    W_bond0 = (W_bond @ W0).astype(np.float32)                # [EDIM, SDIM]
    b_eff = (b_bond @ W0 + b_b0).astype(np.float32)           # [SDIM]

    coords = p + (v @ W_coords).reshape(n, 3)
    coords = coords - coords.reshape(NUM_MOL, ATOMS, 3).mean(1)[batch]

    # half-edge enumeration (j < i), j-major — 496 per molecule
    jl, il = np.triu_indices(ATOMS, k=1)
    fwd_local = jl * 31 + il - 1            # id of (j,i) in the full list
    mir_local = il * 31 + jl                # id of (i,j)

    mols = np.arange(NUM_MOL)
    gj = (mols[:, None] * ATOMS + jl).ravel()         # [NUM_MOL*HP]
    gi = (mols[:, None] * ATOMS + il).ravel()
    d_half = ((coords[gi] - coords[gj]) ** 2).sum(-1).astype(np.float32)
    fwd_ids = (mols[:, None] * (ATOMS * (ATOMS - 1)) + fwd_local).ravel()
    mir_ids = (mols[:, None] * (ATOMS * (ATOMS - 1)) + mir_local).ravel()
    e_sym_half = (0.5 * (e[fwd_ids] + e[mir_ids])).astype(np.float16)
    G_half = (a[gi] + a[gj]
              + d_half[:, None] * w_d[None, :]).astype(np.float16)

    wt = np.zeros((128, 266), np.float16)
    wt[:, 0:256] = W_bond0
    wt[:, 256:261] = W_b1[0:128]
    wt[:, 261:266] = W_b1[128:256]
    bias = np.stack([b_eff[0:128], b_eff[128:256]], axis=1).astype(np.float32)

    in_maps = []
    for c in range(N_CORES):
        eg = np.zeros((128, MOL_PER_CORE * CW), np.float16)
        for mloc in range(MOL_PER_CORE):
            m = c * MOL_PER_CORE + mloc
            b0 = mloc * CW
            esl = slice(m * HP, (m + 1) * HP)
            eg[:, b0:b0 + HP] = e_sym_half[esl].T
            GT = G_half[esl].T                          # [256, HP]
            eg[:, b0 + HP:b0 + 2 * HP] = GT[0:128]
            eg[:, b0 + 2 * HP:b0 + 3 * HP] = GT[128:256]
        in_maps.append({"wt": wt, "bias": bias, "eg": eg})

    try:
        nc = _build_nc()
        # The axon loopback relay can start the NEFF while the host->device
        # input copy is still streaming (first execution reads partially
        # written DRAM). Run until two consecutive executions agree — by the
        # second run the buffers are stable, so this normally costs exactly
        # two executions.
        prev = None
        for _attempt in range(6):
            res = run_bass_kernel_spmd(nc, in_maps,
                                       core_ids=list(range(N_CORES)))
            cur = [res.results[c]["outT"] for c in range(N_CORES)]
            if prev is not None and all(
                    np.array_equal(a, b) for a, b in zip(prev, cur)):
                break
            prev = cur
        globals()["LAST_RES"] = res
        results = res.results if hasattr(res, "results") else res
        out = np.zeros((E, NB), np.float32)
        for c in range(N_CORES):
            oh = results[c]["outT"].T.astype(np.float32) + b_b1  # [E_LOC, NB]
            lo, hi = c * E_LOC, (c + 1) * E_LOC
            out[fwd_ids[lo:hi]] = oh
            out[mir_ids[lo:hi]] = oh
        return out
    except Exception:
        import traceback
        traceback.print_exc()
        return _host_general(s, v, p, e, batch, edge_index,
                             W_shared, b_shared, W_coords, W_bond, b_bond,
                             W_b0, b_b0, W_b1, b_b1)


# revision 33
# speedup vs baseline: 1.0359x; 1.0065x over previous
"""EdgePredictionHead on 8 TRN2 NeuronCores.

Key structure exploited:

1. The reference output is exactly symmetric in (i, j): e_sym, the squared
   distance d, and s[i]+s[j] are all symmetric, so bonds_pred for edge (j,i)
   equals bonds_pred for (i,j). We compute only the 496 half-edges (j<i)
   per molecule and mirror on the host -> halves all device work.

2. The edge MLP's first layer is affine, so the host folds W_b0 into the
   other weights:  pre = e_sym @ (W_bond@W0) + G + b_eff  with
   G = a[i]+a[j]+d*w_d,  a = silu(s@W_shared+b_shared) @ W0.  G is cheap to
   build on the host and ships as fp16; b_eff rides as the scalar-engine
   activation bias.

Device pipeline per 496-edge chunk (one molecule), per 128-row half:

    DVE   copies G.T half into the PSUM bank (fp16 -> fp32)
    PE    matmul accumulates W_bond0_half.T @ esymT on top (start=False)
    ACT   h = silu(psum + b_eff_half)            (PSUM -> SBUF fp16)
    PE    outT[5,496] = wb1a.T @ h0 + wb1b.T @ h1
    DVE/ACT copy outT into an SBUF staging tile; one DMA at the end.

Input DMAs are spread across the SP and GpSimd sequencers (separate DGE
queues) so descriptor issue isn't serialized on one engine.  Everything on
device is fp16 (PSUM accumulates fp32); rel err vs the fp32 reference is
~6e-4, far under the 2e-2 gate.
"""

import os
import sys
import numpy as np

sys.path.insert(0, "/opt/trn_rl_repo")

import concourse.bass as bass
import concourse.mybir as mybir
from concourse import bacc
from concourse.tile import TileContext
from concourse.bass_utils import run_bass_kernel_spmd

N_CORES = 8
SDIM = 256
EDIM = 128
NB = 5
ATOMS = 32
NUM_MOL = 32
MOL_PER_CORE = NUM_MOL // N_CORES          # 4
HP = ATOMS * (ATOMS - 1) // 2              # 496 half-edges per molecule
E_LOC = MOL_PER_CORE * HP                  # 1984 per core
CW = 3 * HP                                # 1488 cols per chunk in `eg`

F32 = mybir.dt.float32
F16 = mybir.dt.float16

_nc_cache = {}


def _build_nc():
    if "nc" in _nc_cache:
        return _nc_cache["nc"]
    nc = bacc.Bacc()
    wt = nc.dram_tensor("wt", [128, 266], F16, kind="ExternalInput")
    bias = nc.dram_tensor("bias", [128, 2], F32, kind="ExternalInput")
    # per chunk [128, 1488]: [esymT 496 | G.T rows 0:128 | G.T rows 128:256]
    eg = nc.dram_tensor("eg", [128, MOL_PER_CORE * CW], F16,
                        kind="ExternalInput")
    outT = nc.dram_tensor("outT", [NB, E_LOC], F32, kind="ExternalOutput")

    Silu = mybir.ActivationFunctionType.Silu
    with TileContext(nc) as tc:
        with tc.tile_pool(name="const", bufs=1) as cpool, \
             tc.tile_pool(name="io", bufs=4) as bpool, \
             tc.tile_pool(name="work", bufs=4) as wpool, \
             tc.tile_pool(name="pp", bufs=2, space="PSUM") as ppool, \
             tc.tile_pool(name="po", bufs=2, space="PSUM") as opool:
            wt_t = cpool.tile([128, 266], F16)
            bias_t = cpool.tile([128, 2], F32)
            out_sb = cpool.tile([NB, E_LOC], F32)

            # Input DMAs spread over the two HWDGE queues (SP / ACT) so the
            # descriptor streams run in parallel.
            eg_ts = []
            for _m in range(MOL_PER_CORE):
                eg_t = bpool.tile([128, CW], F16, tag="eg")
                eg_ts.append(eg_t)
            nc.sync.dma_start(out=eg_ts[0][:], in_=eg[:, 0:CW])
            nc.scalar.dma_start(out=wt_t[:], in_=wt[:])
            nc.scalar.dma_start(out=bias_t[:], in_=bias[:])
            nc.scalar.dma_start(out=eg_ts[1][:], in_=eg[:, CW:2 * CW])
            nc.sync.dma_start(out=eg_ts[2][:], in_=eg[:, 2 * CW:3 * CW])
            nc.scalar.dma_start(out=eg_ts[3][:], in_=eg[:, 3 * CW:4 * CW])

            # one-time warmups: sync ACT/DVE with const DMAs + load Silu table
            wu_a = cpool.tile([128, 1], F32)
            nc.scalar.activation(wu_a[:], bias_t[:, 0:1], Silu,
                                 bias=bias_t[:, 1:2])
            wu_d = cpool.tile([128, 1], F16)
            nc.vector.tensor_copy(wu_d[:], wt_t[:, 0:1])

            hs = {}

            def in_stage(m):
                eg_t = eg_ts[m]
                c0 = 0
                for hf in range(2):
                    ps = ppool.tile([128, HP], F32, tag=f"ps{hf}")
                    nc.vector.tensor_copy(
                        ps[:],
                        eg_t[:, c0 + HP + hf * HP:c0 + HP + (hf + 1) * HP])
                    nc.tensor.matmul(
                        ps[:], wt_t[:, hf * 128:(hf + 1) * 128],
                        eg_t[:, c0:c0 + HP], start=False, stop=True,
                        skip_group_check=True)
                    h_t = wpool.tile([128, HP], F16, tag=f"h{hf}")
                    nc.scalar.activation(h_t[:], ps[:], Silu,
                                         bias=bias_t[:, hf:hf + 1])
                    hs[(m, hf)] = h_t

            def out_stage(m):
                psO = opool.tile([128, HP], F32, tag="o")
                nc.tensor.matmul(psO[0:NB, :], wt_t[:, 256:261],
                                 hs[(m, 0)][:], start=True, stop=False)
                nc.tensor.matmul(psO[0:NB, :], wt_t[:, 261:266],
                                 hs[(m, 1)][:], start=False, stop=True)
                sl = slice(m * HP, (m + 1) * HP)
                nc.vector.tensor_copy(out_sb[:, sl], psO[0:NB, :])

            # software-pipelined emission: chunk m's input stage is emitted
            # before chunk m-1's output stage, so no engine's in-order
            # stream stalls on a cross-stage dependency. Output DMAs go out
            # in two halves so the first fires mid-kernel.
            for m in range(MOL_PER_CORE):
                in_stage(m)
                if m >= 1:
                    out_stage(m - 1)
            out_stage(MOL_PER_CORE - 1)
            nc.sync.dma_start(out=outT[:], in_=out_sb[:])

    nc.finalize()   # Bacc compile pipeline: event-sem split, act tables, ...

    # Drop the vestigial set-0 act-table load the table pass puts at block
    # entry: every activation here is Silu, whose own load follows before
    # the first InstActivation. Saves a 1.28us ACT_TABLE_LOAD.
    for b in nc.m.functions[0].blocks:
        insts = list(b.instructions)
        loads = [i for i in insts
                 if type(i).__name__ == 'InstLoadActFuncSet']
        if len(loads) >= 2 and not loads[0].sync_info:
            k1 = insts.index(loads[1])
            first_act = next((k for k, i in enumerate(insts)
                              if type(i).__name__ == 'InstActivation'),
                             None)
            if first_act is not None and k1 < first_act:
                b.instructions = [i for i in insts if i is not loads[0]]

    _nc_cache["nc"] = nc
    return nc


def _silu(x):
    return x / (1.0 + np.exp(-x))


def _structured(batch, edge_index, n, E):
    """True iff inputs follow the fully-connected per-molecule pattern."""
    if n != NUM_MOL * ATOMS or E != NUM_MOL * ATOMS * (ATOMS - 1):
        return False
    if not np.array_equal(batch,
                          np.repeat(np.arange(NUM_MOL, dtype=batch.dtype),
                                    ATOMS)):
        return False
    jj, ii = np.meshgrid(np.arange(ATOMS), np.arange(ATOMS), indexing="ij")
    msk = jj != ii
    offs = (np.arange(NUM_MOL) * ATOMS)[:, None]
    exp_j = (jj[msk][None, :] + offs).ravel()
    exp_i = (ii[msk][None, :] + offs).ravel()
    return (np.array_equal(edge_index[0], exp_j)
            and np.array_equal(edge_index[1], exp_i))


def _host_general(s, v, p, e, batch, edge_index,
                  W_shared, b_shared, W_coords, W_bond, b_bond,
                  W_b0, b_b0, W_b1, b_b1):
    """Generic fp32 host path for inputs that don't match the structure."""
    n = s.shape[0]
    E = edge_index.shape[1]
    j, i = edge_index[0].astype(np.int64), edge_index[1].astype(np.int64)
    s1 = _silu(s @ W_shared + b_shared)
    coords = p + (v @ W_coords).reshape(n, 3)
    nmol = int(batch.max()) + 1
    sums = np.zeros((nmol, 3), np.float32)
    np.add.at(sums, batch, coords)
    counts = np.maximum(np.bincount(batch, minlength=nmol), 1).astype(np.float32)
    coords = coords - (sums / counts[:, None])[batch]
    d = ((coords[i] - coords[j]) ** 2).sum(-1).astype(np.float32)
    key = j * n + i
    order = np.argsort(key)
    skey = key[order]
    pos = np.clip(np.searchsorted(skey, i * n + j), 0, E - 1)
    rev = order[pos]
    has_rev = skey[pos] == i * n + j
    e_rev = np.where(has_rev[:, None], e[rev], 0.0).astype(np.float32)
    e_sym = 0.5 * (e + e_rev)
    f = s1[i] + s1[j] + (e_sym @ W_bond + b_bond)
    h = _silu(np.concatenate([f, d[:, None]], -1) @ W_b0 + b_b0)
    return (h @ W_b1 + b_b1).astype(np.float32)


def kernel(s, v, p, e, batch, edge_index,
           W_shared, b_shared, W_coords, W_bond, b_bond,
           W_b0, b_b0, W_b1, b_b1):
    s = np.asarray(s, np.float32)
    v = np.asarray(v, np.float32)
    p = np.asarray(p, np.float32)
    e = np.asarray(e, np.float32)
    batch = np.asarray(batch, np.int32)
    edge_index = np.asarray(edge_index, np.int32)
    W_shared = np.asarray(W_shared, np.float32)
    b_shared = np.asarray(b_shared, np.float32)
    W_coords = np.asarray(W_coords, np.float32)
    W_bond = np.asarray(W_bond, np.float32)
    b_bond = np.asarray(b_bond, np.float32)
    W_b0 = np.asarray(W_b0, np.float32)
    b_b0 = np.asarray(b_b0, np.float32)
    W_b1 = np.asarray(W_b1, np.float32)
    b_b1 = np.asarray(b_b1, np.float32)
    n = s.shape[0]
    E = edge_index.shape[1]

    if not _structured(batch, edge_index, n, E):
        return _host_general(s, v, p, e, batch, edge_index,
                             W_shared, b_shared, W_coords, W_bond, b_bond,
                             W_b0, b_b0, W_b1, b_b1)

    # ---- host: node-level prep (cheap) + weight folding ----
    s1 = _silu(s @ W_shared + b_shared)                       # [n, SDIM]
    W0 = W_b0[:SDIM]                                          # [SDIM, SDIM]
    w_d = W_b0[SDIM]                                          # [SDIM]
    a = (s1 @ W0).astype(np.float32)                          # [n, SDIM]
    W_bond0 = (W_bond @ W0).astype(np.float32)                # [EDIM, SDIM]
    b_eff = (b_bond @ W0 + b_b0).astype(np.float32)           # [SDIM]

    coords = p + (v @ W_coords).reshape(n, 3)
    coords = coords - coords.reshape(NUM_MOL, ATOMS, 3).mean(1)[batch]

    # half-edge enumeration (j < i), j-major — 496 per molecule
    jl, il = np.triu_indices(ATOMS, k=1)
    fwd_local = jl * 31 + il - 1            # id of (j,i) in the full list
    mir_local = il * 31 + jl                # id of (i,j)

    mols = np.arange(NUM_MOL)
    gj = (mols[:, None] * ATOMS + jl).ravel()         # [NUM_MOL*HP]
    gi = (mols[:, None] * ATOMS + il).ravel()
    d_half = ((coords[gi] - coords[gj]) ** 2).sum(-1).astype(np.float32)
    fwd_ids = (mols[:, None] * (ATOMS * (ATOMS - 1)) + fwd_local).ravel()
    mir_ids = (mols[:, None] * (ATOMS * (ATOMS - 1)) + mir_local).ravel()
    e_sym_half = (0.5 * (e[fwd_ids] + e[mir_ids])).astype(np.float16)
    G_half = (a[gi] + a[gj]
              + d_half[:, None] * w_d[None, :]).astype(np.float16)

    wt = np.zeros((128, 266), np.float16)
    wt[:, 0:256] = W_bond0
    wt[:, 256:261] = W_b1[0:128]
    wt[:, 261:266] = W_b1[128:256]
    bias = np.stack([b_eff[0:128], b_eff[128:256]], axis=1).astype(np.float32)

    in_maps = []
    for c in range(N_CORES):
        eg = np.zeros((128, MOL_PER_CORE * CW), np.float16)
        for mloc in range(MOL_PER_CORE):
            m = c * MOL_PER_CORE + mloc
            b0 = mloc * CW
            esl = slice(m * HP, (m + 1) * HP)
            eg[:, b0:b0 + HP] = e_sym_half[esl].T
            GT = G_half[esl].T                          # [256, HP]
            eg[:, b0 + HP:b0 + 2 * HP] = GT[0:128]
            eg[:, b0 + 2 * HP:b0 + 3 * HP] = GT[128:256]
        in_maps.append({"wt": wt, "bias": bias, "eg": eg})

    try:
        nc = _build_nc()
        # The axon loopback relay can start the NEFF while the host->device
        # input copy is still streaming (first execution reads partially
        # written DRAM). Run until two consecutive executions agree — by the
        # second run the buffers are stable, so this normally costs exactly
        # two executions.
        prev = None
        for _attempt in range(6):
            res = run_bass_kernel_spmd(nc, in_maps,
                                       core_ids=list(range(N_CORES)))
            cur = [res.results[c]["outT"] for c in range(N_CORES)]
            if prev is not None and all(
                    np.array_equal(a, b) for a, b in zip(prev, cur)):
                break
            prev = cur
        globals()["LAST_RES"] = res
        results = res.results if hasattr(res, "results") else res
        out = np.zeros((E, NB), np.float32)
        for c in range(N_CORES):
            oh = results[c]["outT"].T.astype(np.float32) + b_b1  # [E_LOC, NB]
            lo, hi = c * E_LOC, (c + 1) * E_LOC
            out[fwd_ids[lo:hi]] = oh
            out[mir_ids[lo:hi]] = oh
        return out
    except Exception:
        import traceback
        traceback.print_exc()
        return _host_general(s, v, p, e, batch, edge_index,
                             W_shared, b_shared, W_coords, W_bond, b_bond,
                             W_b0, b_b0, W_b1, b_b1)
